# revision 80
# baseline (speedup 1.0000x reference)
"""MoE layer (GShard top-2 routing + per-expert FFN) on 8 Trainium2 NeuronCores.

Strategy (expert parallelism, fp8-DoubleRow FFN, split ReduceScatter combine):
  - Router matmul (fp32, exact) is token-sharded: each core computes logits for
    its 1024-token shard, then an AllGather shares per-token routing scalars
    (idx1, idx2, g1/WSC, g2/WSC). The payload layout (h tt l a) pairs with the
    xT column perm so the all-core reread is ONE affine DMA.
  - Every core replicates the (cheap) global slot-assignment math: per-expert
    inclusive scans along the free dim + a triangular-matmul partition prefix
    give each token its capacity slot exactly as the reference's cumsum does.
  - Each core owns ONE expert. The slot->token map is built with local_scatter
    (per-partition scatter of token ids by slot), merged across partitions
    with a gpsimd partition all-reduce (each slot column has one writer), and
    read out column-major via a diagonal extraction (first 4 columns early so
    cb0's dispatch gathers start while the rest extract).
  - Dispatch: 16 indirect row gathers from x (bf16) + PE transposes (dt-major)
    give the [d, slot] layout, then an Act/DVE pair splits it into fp8 hi/lo
    (xh = fp8(x), xl = fp8(x - xh)).
  - FFN in fp8 e4m3 with DoubleRow perf mode (2 K-blocks per instruction, 0.5
    cycles/row = 4x bf16 K-throughput). Weights ship as host-quantized scaled
    hi/lo pairs (wh = fp8(WSC*w), wl = fp8(WSC*w - wh); w ~N(0,0.02) sits
    below e4m3's min normal, hence the WSC=512 scale). Each matmul runs 3
    DR terms — ah@bh + ah@bl + al@bh — which restores ~bf16 accuracy at 0.75x
    bf16 cycle cost: PSUM = WSC*(x@w_gate); gelu applies scale=1/WSC (bf16
    out), DVE derives the fp8 h hi/lo pair for mm2; mm2's PSUM carries WSC,
    undone by the slot gate that was pre-divided by WSC in the payload.
  - Combine via two ReduceScatters over token-space partial buffers split by
    columns: partA ([T+1, DA]) and partB ([T+1, DB]), both bf16 and
    zero-filled on device (collectives cannot touch IO tensors). The slot
    gates arrive by gathering the payload table by the slot->token map; mm2's
    PSUM->SBUF copy scales eo rows, which are indirect-scattered as two
    contiguous spans into partA/partB (trash row T for empty slots). The LAST
    mm2 block computes its DA columns for all slots first, so the small
    partA ReduceScatter overlaps the rest of the block; the big partB one is
    bounded by PE-end. y bounces (rsA during RS_B, rsB at the end) finish it.

  Scheduling notes (the TimelineSim cost model serializes all DMA on one
  device, FIFO by acquire time, and Tile list-schedules by model-ready time
  then emission priority — measure every change, intuition fails here):
  - Weight streams run as single-in-flight gated chains. wgt: 8x1MB links,
    head gated on the payload pack, gates on Act (on DVE they crawl behind
    the routing masks/scans and delay the wgtl tail past mm1's first DR
    groups, which read ALL of wgth+wgtl). wdn: 4x2MB links gated in parallel
    on cb0's last dispatch-transpose copy, so they hit the FIFO after cb0's
    gathers and hide under mm1(cb0); both wdn tensors must be resident by
    cb0's mm2 (its DR groups span wdnh+wdnl).
  - The 16.8MB part zero-fill is FULLY deferred into the FFN: a tiny release
    DMA gated on cb0's dispT opens a self-chained 1MB-chunk chain (partA
    then partB) hosted on SP (idle mid-FFN). At most one chunk is in flight,
    so the chain self-yields the FIFO; the eo scatters' WAW on the zero
    windows orders them after their tensor's chunks.
  - Indirect scatters claim a strided static window (rows 0,64,...,8128, cols
    [0:DA)/[0:DB)) of their part tensor: cost is charged on the static AP,
    the actual rows come from the dynamic offsets, and the window must have
    AP offset 0. Do NOT "slice" SBUF partition dims via rearrange in DMA APs
    (e.g. "(r p16) i -> r p16 i") — partition_size silently becomes r and
    the transfer writes garbage on hardware.
  - PSUM accumulation groups must run start->stop with NO other group
    interleaved in the same 2KB bank: start=True marks the whole bank
    pending-zero on hardware (ZERO_REGION_SIZE), silently destroying other
    groups' partials. A kd-outer router loop (8 interleaved tt groups in one
    bank) measured rel_err 0.55 on trn2 while passing the timing sim.
"""

import sys

if "/opt/trn_rl_repo" not in sys.path:
    sys.path.insert(0, "/opt/trn_rl_repo")

import numpy as np
import ml_dtypes

import concourse.bacc as bacc
import concourse.mybir as mybir
import concourse.tile as tile
from concourse import bass
from concourse import bass_isa
from concourse.bass_utils import run_bass_kernel_spmd

BF16 = mybir.dt.bfloat16
F32 = mybir.dt.float32
F8 = mybir.dt.float8e4
I16 = mybir.dt.int16
I32 = mybir.dt.int32
AF = mybir.ActivationFunctionType
OP = mybir.AluOpType
AX = mybir.AxisListType
DR = mybir.MatmulPerfMode.DoubleRow

# fp8 FFN: weights are shipped as scaled hi/lo e4m3 pairs (w ~N(0,0.02) sits
# below e4m3's min normal 2^-6, so the hi part is quantized at 512*w and the
# residual at the same scale; PSUM accumulates 512*(x@w) and the 1/512 folds
# into the gelu input scale / the slot-gate payload).
WSC = 512.0
INV_WSC = 1.0 / WSC

B, S, D, E, F = 4, 2048, 1024, 8, 4096
T = B * S            # 8192 tokens
C = 2 * T // E       # 2048 capacity
NC = 8               # cores
SH = T // NC         # 1024 tokens per shard
CBLK = 512           # FFN slot-block
NCB = C // CBLK      # 4 blocks
ZC = 512             # part zero-fill chunk rows (1MB bf16)
DA = 288             # columns in the small (early) combine half
DB = D - DA          # columns in the big (late) combine half

LAST_RESULT = None   # BassKernelResults of the most recent run (for profiling)


def _build_program():
    nc = bacc.Bacc("TRN2", target_bir_lowering=False, debug=False, num_devices=NC)

    # ---- per-core external inputs ----
    xT_sh = nc.dram_tensor("xT_sh", [D, SH], F32, kind="ExternalInput").ap()
    xb = nc.dram_tensor("xb", [T + 1, D], BF16, kind="ExternalInput").ap()
    wg_d = nc.dram_tensor("wg", [D, E], F32, kind="ExternalInput").ap()
    wgth_d = nc.dram_tensor("wgth", [D, F], F8, kind="ExternalInput").ap()
    wgtl_d = nc.dram_tensor("wgtl", [D, F], F8, kind="ExternalInput").ap()
    wdnh_d = nc.dram_tensor("wdnh", [F, D], F8, kind="ExternalInput").ap()
    wdnl_d = nc.dram_tensor("wdnl", [F, D], F8, kind="ExternalInput").ap()
    cid_d = nc.dram_tensor("cid", [128, 1], F32, kind="ExternalInput").ap()
    slotid_d = nc.dram_tensor("slotid", [128, C // 128], F32, kind="ExternalInput").ap()
    # host-generated constants (gpsimd iota/affine_select aren't available)
    ident_d = nc.dram_tensor("ident", [128, 128], F32, kind="ExternalInput").ap()
    slmat_d = nc.dram_tensor("slmat", [128, 128], F32, kind="ExternalInput").ap()
    tidx_d = nc.dram_tensor("tidx", [128, 64], F32, kind="ExternalInput").ap()
    eidx_d = nc.dram_tensor("eidx", [128, E], F32, kind="ExternalInput").ap()
    carrym_d = nc.dram_tensor("carrym", [128, E * 64], F32, kind="ExternalInput").ap()
    y_d = nc.dram_tensor("y", [SH, D], BF16, kind="ExternalOutput").ap()
    # token-space partial output, split into D-halves so the combine runs as
    # two ReduceScatters with the left one overlapping the last mm2 block;
    # zero-filled on device during the FFN (collectives may not read IO
    # tensors, so these stay internal)
    partA_d = nc.dram_tensor("partA", [T + 1, DA], BF16).ap()
    partB_d = nc.dram_tensor("partB", [T + 1, DB], BF16).ap()

    zsrc_d = nc.dram_tensor("zsrc", [SH, D], BF16, kind="ExternalInput").ap()

    # ---- internal DRAM ----
    pay_in = nc.dram_tensor("pay_in", [4 * SH], F32).ap()
    pay_all = nc.dram_tensor("pay_all", [NC * 4 * SH], F32, addr_space="Shared").ap()
    pay_tab = nc.dram_tensor("pay_tab", [T + 1, 4], F32).ap()
    rsA_out = nc.dram_tensor("rsA", [SH, DA], BF16).ap()
    rsB_out = nc.dram_tensor("rsB", [SH, DB], BF16).ap()

    with tile.TileContext(nc) as tc:
        with tc.tile_pool(name="persist", bufs=1) as pp:
            # route pools are opened here (before the persist consts, so xT's
            # DMA is emitted first) and closed explicitly before the FFN to
            # free their SBUF/PSUM
            _route_cm = tc.tile_pool(name="route", bufs=1)
            pr = _route_cm.__enter__()
            _psum_s_cm = tc.tile_pool(name="psum_s", bufs=2, space="PSUM")
            pss = _psum_s_cm.__enter__()

            # xT is the head of the critical path: emit it before everything
            # else so it gets the first DMA slot (the tt-outer router needs
            # the whole tensor before its first accumulation group anyway)
            xT_sb = pr.tile([128, D // 128, SH], F32)
            nc.sync.dma_start(xT_sb[:], xT_sh.rearrange("(o q) t -> q o t", q=128))
            wg_sb = pr.tile([128, D // 128, E], F32)
            nc.sync.dma_start(wg_sb[:], wg_d.rearrange("(o q) e -> q o e", q=128))



            ident = pp.tile([128, 128], F32)
            nc.sync.dma_start(ident[:], ident_d[:])
            ident_bf = pp.tile([128, 128], BF16)
            nc.vector.tensor_copy(ident_bf[:], ident[:])
            cid = pp.tile([128, 1], F32)
            nc.sync.dma_start(cid[:], cid_d[:])
            slotid = pp.tile([128, C // 128], F32)
            nc.sync.dma_start(slotid[:], slotid_d[:])
            zeros64 = pp.tile([128, 64], F32)
            nc.vector.memset(zeros64[:], 0.0)
            ones128 = pp.tile([128, 128], F32)
            nc.vector.memset(ones128[:], 1.0)

            # resident expert weights (fp8 hi/lo pairs, scaled by WSC)
            wgth_sb = pp.tile([128, D // 128, F], F8)
            wgtl_sb = pp.tile([128, D // 128, F], F8)
            wdnh_sb = pp.tile([128, F // 128, D], F8)
            wdnl_sb = pp.tile([128, F // 128, D], F8)


            # persistent routing products
            tokc = pp.tile([128, C // 128], I32)    # dispatch: slot->token, col-major
            slotg = pp.tile([128, C // 128], F32)   # gate per slot, col-major

            # =============== ROUTER (token shard, fp32) ===============
            if True:
                sl = pr.tile([128, 128], F32)
                nc.sync.dma_start(sl[:], slmat_d[:])
                tif = pr.tile([128, 64], F32)
                nc.sync.dma_start(tif[:], tidx_d[:])
                eidx = pr.tile([128, E], F32)
                nc.sync.dma_start(eidx[:], eidx_d[:])
                carrym = pr.tile([128, E * 64], F32)
                nc.sync.dma_start(carrym[:], carrym_d[:])


                lg = pr.tile([128, 8, E], F32)  # logits, token pos j = 128*tt + p
                psl = pss.tile([128, 8, E], F32, space="PSUM", tag="ps_small")
                # tt OUTER, kd inner: each tt accumulation group runs start ->
                # stop with no other group interleaved. PSUM start=True marks
                # the whole 2KB bank pending-zero (ZERO_REGION_SIZE), so
                # interleaving groups that share a bank silently destroys the
                # other groups' partials ON HARDWARE (kd-outer measured
                # rel_err 0.55 on trn2 while passing the timing sim).
                for tt in range(8):
                    for kd in range(8):
                        nc.tensor.matmul(
                            psl[:, tt, :],
                            lhsT=xT_sb[:, kd, 128 * tt : 128 * tt + 128],
                            rhs=wg_sb[:, kd, :],
                            start=(kd == 0),
                            stop=(kd == 7),
                        )
                nc.vector.tensor_copy(lg[:], psl[:])

                m1x = pr.tile([128, 8], F32)
                nc.vector.tensor_reduce(m1x[:], lg[:], AX.X, OP.max)

                is1 = pr.tile([128, 8, E], F32)
                nc.vector.tensor_tensor(
                    out=is1[:], in0=lg[:], in1=m1x[:, :, None].to_broadcast([128, 8, E]),
                    op=OP.is_equal,
                )
                l2 = pr.tile([128, 8, E], F32)
                nc.vector.scalar_tensor_tensor(
                    out=l2[:], in0=is1[:], scalar=-1e30, in1=lg[:], op0=OP.mult, op1=OP.add,
                )
                m2x = pr.tile([128, 8], F32)
                nc.vector.tensor_reduce(m2x[:], l2[:], AX.X, OP.max)
                is2 = pr.tile([128, 8, E], F32)
                nc.vector.tensor_tensor(
                    out=is2[:], in0=l2[:], in1=m2x[:, :, None].to_broadcast([128, 8, E]),
                    op=OP.is_equal,
                )

                # argmax index = sum(mask * eidx) along E
                i1f = pr.tile([128, 8], F32)
                sc1a = pr.tile([128, 8, E], F32, tag="am_scr_a")
                nc.vector.tensor_tensor(
                    out=sc1a[:], in0=is1[:], in1=eidx[:, None, :].to_broadcast([128, 8, E]),
                    op=OP.mult,
                )
                nc.vector.tensor_reduce(i1f[:], sc1a[:], AX.X, OP.add)
                i2f = pr.tile([128, 8], F32)
                sc2a = pr.tile([128, 8, E], F32, tag="am_scr_b")
                nc.vector.tensor_tensor(
                    out=sc2a[:], in0=is2[:], in1=eidx[:, None, :].to_broadcast([128, 8, E]),
                    op=OP.mult,
                )
                nc.vector.tensor_reduce(i2f[:], sc2a[:], AX.X, OP.add)

                # top-2 softmax gates: g1 = 1/(1+exp(m2-m1)), g2 = 1-g1
                dm = pr.tile([128, 8], F32)
                nc.vector.tensor_tensor(out=dm[:], in0=m2x[:], in1=m1x[:], op=OP.subtract)
                e2 = pr.tile([128, 8], F32)
                nc.scalar.activation(e2[:], dm[:], AF.Exp)
                den = pr.tile([128, 8], F32)
                nc.vector.tensor_scalar_add(den[:], e2[:], 1.0)
                g1 = pr.tile([128, 8], F32)
                nc.vector.reciprocal(g1[:], den[:])
                g2 = pr.tile([128, 8], F32)
                nc.vector.tensor_tensor(out=g2[:], in0=e2[:], in1=g1[:], op=OP.mult)

                pk = pr.tile([128, 8, 4], F32)
                nc.vector.tensor_copy(pk[:, :, 0], i1f[:])
                nc.vector.tensor_copy(pk[:, :, 1], i2f[:])
                # gates pre-scaled by 1/WSC: the slot-gate multiply then undoes
                # the WSC factor carried by the fp8 mm2 PSUM
                nc.vector.tensor_scalar(out=pk[:, :, 2], in0=g1[:], scalar1=INV_WSC, scalar2=None, op0=OP.mult)
                nc.vector.tensor_scalar(out=pk[:, :, 3], in0=g2[:], scalar1=INV_WSC, scalar2=None, op0=OP.mult)
                # payload layout (h tt l a): local token u = 512h + 64tt + l
                # for partition p = 64h + l. Together with the matching xT
                # perm this makes the all-core reread a SINGLE affine DMA
                # (pay_all[256p + 4i + a] = value_a(token 64p + i)).
                pay_v = pay_in.rearrange("(h tt l a) -> h l tt a", h=2, tt=8, l=64)
                for hh in range(2):
                    nc.sync.dma_start(pay_v[hh], pk[64 * hh : 64 * hh + 64, :, :])

                # gate_proj weights (hi then lo): 1MB links, each gated on the
                # previous via a tiny DVE copy; the chain head hangs off the
                # payload pack so the stream starts right after the (critical)
                # payload write, and the small links keep the worst-case FIFO
                # wait for routing-critical DMAs (the AG reread) under ~3us
                wgth_v = wgth_d.rearrange("(o q) f -> q o f", q=128)
                wgtl_v = wgtl_d.rearrange("(o q) f -> q o f", q=128)
                wgt_links = [(wgth_sb, wgth_v, q) for q in range(4)] + \
                            [(wgtl_sb, wgtl_v, q) for q in range(4)]
                prev_sb, prev_off = None, 0
                for sb, v, q in wgt_links:
                    # chain gates live on Act (idle until mm1's gelus): on DVE
                    # they crawl behind the routing masks/scans/diag work and
                    # delay the wgtl tail past mm1's first DR groups
                    src = pk[0:1, 3, 0:2] if prev_sb is None else prev_sb[0:1, prev_off, 0:2]
                    nc.scalar.activation(sb[0:1, 2 * q, 0:2], src, AF.Copy)
                    nc.sync.dma_start(
                        sb[:, 2 * q : 2 * (q + 1), :], v[:, 2 * q : 2 * (q + 1), :]
                    )
                    prev_sb, prev_off = sb, 2 * q

                nc.gpsimd.collective_compute(
                    "AllGather", OP.bypass,
                    replica_groups=[list(range(NC))],
                    ins=[pay_in[:].opt()], outs=[pay_all[:].opt()],
                )

                nc.scalar.dma_start(pay_tab[T : T + 1, :], zeros64[0:1, 0:4])

                # reread all 4 arrays into global routing layout [128, 64]
                # (t = 64p + i). The (h tt a l) payload layout makes the
                # all-core view affine in the partition dim (stride 256), so
                # this is ONE DMA with the SBUF side a plain [128, 4, 64]
                # tile — no SBUF partition-dim rearrange (which would silently
                # drop partition semantics and write garbage on hardware).
                rt = pr.tile([128, 64, 4], F32)
                nc.sync.dma_start(rt[:], pay_all.rearrange("(p i a) -> p i a", p=NC * 16, i=64))
                i1r, i2r = rt[:, :, 0], rt[:, :, 1]
                g1r, g2r = rt[:, :, 2], rt[:, :, 3]


                # =============== SLOT ASSIGNMENT (replicated) ===============
                # (the real compiler only allows generic vector ops on DVE,
                # so both choice chains share it)
                v1, v2 = nc.vector, nc.vector
                m1 = pr.tile([128, E, 64], F32)
                m2 = pr.tile([128, E, 64], F32)
                sc1 = pr.tile([128, E, 64], F32)
                sc2 = pr.tile([128, E, 64], F32)
                v1.tensor_tensor(
                    out=m1[:], in0=i1r[:, None, :].to_broadcast([128, E, 64]),
                    in1=eidx[:, :, None].to_broadcast([128, E, 64]), op=OP.is_equal,
                )
                v2.tensor_tensor(
                    out=m2[:], in0=i2r[:, None, :].to_broadcast([128, E, 64]),
                    in1=eidx[:, :, None].to_broadcast([128, E, 64]), op=OP.is_equal,
                )
                # ONE segmented scan per choice: state = carry*state + m with
                # carry=0 at each expert's first column resets the recurrence
                # at segment boundaries (replaces 8 per-expert scans)
                v1.tensor_tensor_scan(
                    sc1[:].rearrange("p e i -> p (e i)"), carrym[:],
                    m1[:].rearrange("p e i -> p (e i)"), 0.0, op0=OP.mult, op1=OP.add,
                )
                v2.tensor_tensor_scan(
                    sc2[:].rearrange("p e i -> p (e i)"), carrym[:],
                    m2[:].rearrange("p e i -> p (e i)"), 0.0, op0=OP.mult, op1=OP.add,
                )
                tot1 = pr.tile([128, E], F32)
                tot2 = pr.tile([128, E], F32)
                v1.tensor_copy(tot1[:], sc1[:, :, 63])
                v2.tensor_copy(tot2[:], sc2[:, :, 63])

                of1_ps = pss.tile([128, E], F32, space="PSUM", tag="ps_small")
                nc.tensor.matmul(of1_ps[:], lhsT=sl[:], rhs=tot1[:], start=True, stop=True)
                of1 = pr.tile([128, E], F32)
                nc.vector.tensor_scalar_add(of1[:], of1_ps[:], -1.0)
                of2_ps = pss.tile([128, E], F32, space="PSUM", tag="ps_small")
                nc.tensor.matmul(of2_ps[:], lhsT=sl[:], rhs=tot2[:], start=True, stop=False)
                nc.tensor.matmul(of2_ps[:], lhsT=ones128[:], rhs=tot1[:], start=False, stop=True)
                of2 = pr.tile([128, E], F32)
                nc.vector.tensor_scalar_add(of2[:], of2_ps[:], -1.0)

                def loc_s(vv, sc, m, of, tag):
                    # (sc + of) broadcast-added per expert, masked, then a
                    # strided reduce over the expert dim — 3 wide ops instead
                    # of 8 scalar ops + a 3-level tree
                    tmp = pr.tile([128, E, 64], F32, tag=f"loc_tmp{tag}")
                    vv.tensor_tensor(
                        out=tmp[:], in0=sc[:, :, :],
                        in1=of[:, :, None].to_broadcast([128, E, 64]), op=OP.add,
                    )
                    vv.tensor_tensor(out=tmp[:], in0=tmp[:], in1=m[:, :, :], op=OP.mult)
                    ls = pr.tile([128, 64], F32, tag=f"loc_ls{tag}")
                    vv.tensor_reduce(ls[:], tmp[:].rearrange("p e i -> p i e"), AX.X, OP.add)
                    return ls[:]

                l1s = loc_s(v1, sc1, m1, of1, "a")
                l2s = loc_s(v2, sc2, m2, of2, "b")

                def keep_f(vv, ls, ir, tag):
                    kp = pr.tile([128, 64], F32, tag=f"kp{tag}")
                    vv.tensor_scalar(out=kp[:], in0=ls, scalar1=float(C), scalar2=None, op0=OP.is_lt)
                    lc = pr.tile([128, 64], F32, tag=f"lc{tag}")
                    vv.tensor_scalar(out=lc[:], in0=ls, scalar1=float(C - 1), scalar2=None, op0=OP.min)
                    f = pr.tile([128, 64], F32, tag=f"f{tag}")
                    vv.scalar_tensor_tensor(out=f[:], in0=ir, scalar=float(C), in1=lc[:], op0=OP.mult, op1=OP.add)
                    return f, kp

                f1, kp1 = keep_f(v1, l1s, i1r, "a")
                f2, kp2 = keep_f(v2, l2s, i2r, "b")

                # payload table rows t = 64p + i: (f1, f2, g1, g2)
                pt_sb = pr.tile([128, 64, 4], F32)
                nc.vector.tensor_copy(pt_sb[:, :, 0], f1[:])
                nc.vector.tensor_copy(pt_sb[:, :, 1], f2[:])
                nc.vector.tensor_copy(pt_sb[:, :, 2], g1r)
                nc.vector.tensor_copy(pt_sb[:, :, 3], g2r)
                nc.sync.dma_start(
                    pay_tab[0:T, :].rearrange("(p i) c -> p i c", p=128), pt_sb[:]
                )

                # ====== SLOT -> TOKEN MAP (local_scatter + merge + diagonal) ======
                tp1 = pr.tile([128, 64], F32)
                nc.vector.tensor_scalar_add(tp1[:], tif[:], 1.0)   # token id + 1

                def slot_halves(vv, ls, ir, kp, tag):
                    # sel = (expert == cid) && kept; slot+1 where selected else 0
                    isc = pr.tile([128, 64], F32, tag=f"isc{tag}")
                    vv.tensor_tensor(out=isc[:], in0=ir, in1=cid[:, 0:1].to_broadcast([128, 64]), op=OP.is_equal)
                    sel = pr.tile([128, 64], F32, tag=f"sel{tag}")
                    vv.tensor_tensor(out=sel[:], in0=isc[:], in1=kp[:], op=OP.mult)
                    sp1 = pr.tile([128, 64], F32, tag=f"sp1{tag}")  # sel ? slot+1 : 0
                    vv.tensor_scalar_add(sp1[:], ls, 1.0)
                    vv.tensor_tensor(out=sp1[:], in0=sp1[:], in1=sel[:], op=OP.mult)
                    # lo half: slot in [0, 1024): idx = slot, else -1
                    mlo = pr.tile([128, 64], F32, tag=f"mlo{tag}")
                    vv.tensor_scalar(out=mlo[:], in0=sp1[:], scalar1=1024.0, scalar2=None, op0=OP.is_le)
                    vv.tensor_tensor(out=mlo[:], in0=mlo[:], in1=sel[:], op=OP.mult)
                    ilo = pr.tile([128, 64], F32, tag=f"ilo{tag}")
                    vv.tensor_tensor(out=ilo[:], in0=mlo[:], in1=sp1[:], op=OP.mult)
                    vv.tensor_scalar_add(ilo[:], ilo[:], -1.0)
                    # hi half: slot in [1024, 2048): idx = slot - 1024, else -1
                    mhi = pr.tile([128, 64], F32, tag=f"mhi{tag}")
                    vv.tensor_scalar(out=mhi[:], in0=sp1[:], scalar1=1024.0, scalar2=None, op0=OP.is_gt)
                    ihi = pr.tile([128, 64], F32, tag=f"ihi{tag}")
                    vv.tensor_scalar_add(ihi[:], sp1[:], -1024.0)
                    vv.tensor_tensor(out=ihi[:], in0=ihi[:], in1=mhi[:], op=OP.mult)
                    vv.tensor_scalar_add(ihi[:], ihi[:], -1.0)
                    return ilo, ihi

                i1lo, i1hi = slot_halves(v1, l1s, i1r, kp1, "a")
                i2lo, i2hi = slot_halves(v2, l2s, i2r, kp2, "b")

                data128 = pr.tile([128, 128], I16)
                v1.tensor_copy(data128[:, :64], tp1[:])
                v2.tensor_copy(data128[:, 64:], tp1[:])
                idxlo = pr.tile([128, 128], I16)
                v1.tensor_copy(idxlo[:, :64], i1lo[:])
                v2.tensor_copy(idxlo[:, 64:], i2lo[:])
                idxhi = pr.tile([128, 128], I16)
                v1.tensor_copy(idxhi[:, :64], i1hi[:])
                v2.tensor_copy(idxhi[:, 64:], i2hi[:])

                dst_lo = pr.tile([128, 1024], I16)
                nc.gpsimd.local_scatter(dst_lo[:], data128[:], idxlo[:], channels=128, num_elems=1024, num_idxs=128)
                dst_hi = pr.tile([128, 1024], I16)
                nc.gpsimd.local_scatter(dst_hi[:], data128[:], idxhi[:], channels=128, num_elems=1024, num_idxs=128)

                # merge across partitions: each slot column has at most one
                # nonzero writer, so a gpsimd partition all-reduce (max)
                # replicates the slot->token map onto every partition
                merged = pr.tile([128, 2, 1024], F32)  # map+1 on all partitions
                nc.gpsimd.partition_all_reduce(
                    merged[:, 0, :], dst_lo[:], channels=128, reduce_op=bass_isa.ReduceOp.max
                )
                nc.gpsimd.partition_all_reduce(
                    merged[:, 1, :], dst_hi[:], channels=128, reduce_op=bass_isa.ReduceOp.max
                )

                # diagonal extraction: tokraw[p, k] = merged-flat[128k + p]
                tokraw = pr.tile([128, C // 128], F32)
                scratch = pr.tile([128, 128], F32, tag="diag_scr")
                mview = merged[:].rearrange("p a b -> p (a b)")
                scratch2 = pr.tile([128, 128], F32, tag="diag_scr2")
                iszero = pr.tile([128, C // 128], F32)

                def diag_cols(k0, k1):
                    # extract columns [k0,k1), sanitize (0 -> T+1; v -> v-1),
                    # and publish them to tokc so dependent gathers can start
                    for k in range(k0, k1):
                        vv, scr = (v1, scratch) if k % 2 == 0 else (v2, scratch2)
                        vv.scalar_tensor_tensor(
                            out=scr[:], in0=mview[:, 128 * k : 128 * (k + 1)], scalar=0.0,
                            in1=ident[:], op0=OP.add, op1=OP.mult,
                            accum_out=tokraw[:, k : k + 1],
                        )
                    nc.vector.tensor_scalar(out=iszero[:, k0:k1], in0=tokraw[:, k0:k1], scalar1=0.0, scalar2=None, op0=OP.is_equal)
                    nc.vector.scalar_tensor_tensor(
                        out=tokraw[:, k0:k1], in0=iszero[:, k0:k1], scalar=float(T + 1),
                        in1=tokraw[:, k0:k1], op0=OP.mult, op1=OP.add,
                    )
                    nc.vector.tensor_scalar_add(tokraw[:, k0:k1], tokraw[:, k0:k1], -1.0)
                    nc.vector.tensor_copy(tokc[:, k0:k1], tokraw[:, k0:k1])

                # cb0's dispatch gathers need only the first 4 columns: emit
                # them first so the FFN pipeline starts while the rest extract
                diag_cols(0, CBLK // 128)
                diag_cols(CBLK // 128, C // 128)

            _psum_s_cm.__exit__(None, None, None)
            _route_cm.__exit__(None, None, None)

            # =============== EXPERT FFN (fp8 DoubleRow, hi/lo) ===============
            with (
                tc.tile_pool(name="ffn", bufs=1) as pf,
                tc.tile_pool(name="ffn_db", bufs=2) as pfd,
                tc.tile_pool(name="ffn_d1", bufs=1) as pf1,
                tc.tile_pool(name="ffn_dr", bufs=4) as pdr,
                tc.tile_pool(name="psum_mm", bufs=2, space="PSUM") as psm,
            ):
                KT = CBLK // 128

                def emit_gathers(cb):
                    # gather 4 x 128 slot rows from the token table
                    drows = []
                    for kt in range(KT):
                        k = KT * cb + kt
                        drow = pdr.tile([128, D], BF16, tag="drow")
                        nc.gpsimd.indirect_dma_start(
                            out=drow[:], out_offset=None, in_=xb[:],
                            in_offset=bass.IndirectOffsetOnAxis(ap=tokc[:, k : k + 1], axis=0),
                        )
                        drows.append(drow)
                    return drows

                def emit_transpose_batch(drows, dispT, dt):
                    # one dt row of PE transposes + their PSUM drains.
                    # (xbar DMA transposes were tried here — semantically
                    # correct and cheap on paper, but they queue behind the
                    # weight/zero chains on the single FIFO DMA device and
                    # lose 11-26us; PE transposes overlap cleanly.)
                    for kt in range(KT):
                        tr_ps = psm.tile([128, 128], BF16, space="PSUM", tag="ps_tr")
                        nc.tensor.transpose(tr_ps[:], drows[kt][:, 128 * dt : 128 * (dt + 1)], ident_bf[:])
                        nc.vector.tensor_copy(dispT[:, dt, 128 * kt : 128 * (kt + 1)], tr_ps[:])

                def emit_dispatch(cb):
                    # transposes run dt-major so the low kd rows of dispT
                    # complete first — emit_split's kd-chunked DVE ops then
                    # chase them and mm1's first DR groups start early
                    dispT = pf1.tile([128, D // 128, CBLK], BF16, tag="dispT")
                    drows = emit_gathers(cb)
                    for dt in range(D // 128):
                        emit_transpose_batch(drows, dispT, dt)
                    return dispT

                def emit_split(dispT):
                    # hi/lo fp8 split of the dispatched tokens: xh = fp8(x),
                    # xl = fp8(x - xh). Every mm1 DR group reads ALL kd chunks
                    # of dh/dl, so the split is on cb0's critical path: the dh
                    # casts run on Act (idle before the gelus) while DVE does
                    # the dl subtracts in parallel, halving the latency.
                    dh = pfd.tile([128, D // 128, CBLK], F8, tag="dispT_h")
                    dl = pfd.tile([128, D // 128, CBLK], F8, tag="dispT_l")
                    for c4 in range(4):
                        sl4 = slice(2 * c4, 2 * c4 + 2)
                        nc.scalar.activation(dh[:, sl4, :], dispT[:, sl4, :], AF.Copy)
                    for c4 in range(4):
                        sl4 = slice(2 * c4, 2 * c4 + 2)
                        nc.vector.tensor_tensor(
                            out=dl[:, sl4, :], in0=dispT[:, sl4, :], in1=dh[:, sl4, :],
                            op=OP.subtract,
                        )
                    return dh, dl

                next_dispT = emit_dispatch(0)

                # down_proj weights: all 4 links gated (in parallel, on Act's
                # queue) on the slot map, so the 8MB stream hits the DMA FIFO
                # right after cb0's dispatch gathers are requested and hides
                # under mm1(cb0). mm2's DR accumulation groups span all of
                # wdnh+wdnl, so both must be resident by cb0's mm2.
                wdn_links = [
                    (wdnh_sb, wdnh_d.rearrange("(o q) d -> q o d", q=128), 0),
                    (wdnh_sb, wdnh_d.rearrange("(o q) d -> q o d", q=128), 1),
                    (wdnl_sb, wdnl_d.rearrange("(o q) d -> q o d", q=128), 0),
                    (wdnl_sb, wdnl_d.rearrange("(o q) d -> q o d", q=128), 1),
                ]
                for sb, v, hf in wdn_links:
                    nc.scalar.activation(sb[0:1, 16 * hf, 0:2], next_dispT[0:1, 7, 504:506], AF.Copy)
                    nc.sync.dma_start(
                        sb[:, 16 * hf : 16 * (hf + 1), :], v[:, 16 * hf : 16 * (hf + 1), :]
                    )

                for cb in range(NCB):
                    dh, dl = emit_split(next_dispT)

                    if cb == 0:
                        # slot gates: gather payload rows by slot owner, then
                        # gate = (f1==slot)*g1 + (f2==slot)*g2. Emitted after
                        # cb0's dispatch gathers so they don't delay the FFN
                        # start on the (in-order) gpsimd queue; results are
                        # only needed by cb0's mm2 scale, ~100us later.
                        pg = pf.tile([128, C // 128, 4], F32, tag="pg")
                        for k in range(C // 128):
                            nc.gpsimd.indirect_dma_start(
                                out=pg[:, k, :], out_offset=None, in_=pay_tab[:],
                                in_offset=bass.IndirectOffsetOnAxis(ap=tokc[:, k : k + 1], axis=0),
                            )
                        is1g = pf.tile([128, C // 128], F32, tag="is1g")
                        nc.vector.tensor_tensor(out=is1g[:], in0=pg[:, :, 0], in1=slotid[:], op=OP.is_equal)
                        is2g = pf.tile([128, C // 128], F32, tag="is2g")
                        nc.vector.tensor_tensor(out=is2g[:], in0=pg[:, :, 1], in1=slotid[:], op=OP.is_equal)
                        ga = pf.tile([128, C // 128], F32, tag="ga")
                        nc.vector.tensor_tensor(out=ga[:], in0=is1g[:], in1=pg[:, :, 2], op=OP.mult)
                        gb = pf.tile([128, C // 128], F32, tag="gb")
                        nc.vector.tensor_tensor(out=gb[:], in0=is2g[:], in1=pg[:, :, 3], op=OP.mult)
                        nc.vector.tensor_tensor(out=slotg[:], in0=ga[:], in1=gb[:], op=OP.add)

                    # mm1: 3 DR-fp8 terms (xh@wh + xh@wl + xl@wh), PSUM holds
                    # WSC*(x@w_gate); gelu applies the 1/WSC input scale.
                    # Gelu runs twice (fp8 + bf16 out) so DVE can form the h
                    # residual hl = h - hh for mm2's correction term.
                    hT_h = pf.tile([128, F // 128, CBLK], F8, tag="hT_h")
                    hT_l = pf.tile([128, F // 128, CBLK], F8, tag="hT_l")
                    for ft in range(F // 128):
                        ps1 = psm.tile([128, CBLK], F32, space="PSUM", tag="ps1")
                        kk = 0
                        for wsb, dsb in ((wgth_sb, dh), (wgtl_sb, dh), (wgth_sb, dl)):
                            for kd2 in range(D // 256):
                                nc.tensor.matmul(
                                    ps1[:],
                                    lhsT=wsb[:, 2 * kd2 : 2 * kd2 + 2, 128 * ft : 128 * ft + 128],
                                    rhs=dsb[:, 2 * kd2 : 2 * kd2 + 2, :],
                                    start=(kk == 0), stop=(kk == 3 * (D // 256) - 1),
                                    perf_mode=DR,
                                )
                                kk += 1
                        # ONE gelu on Act (bf16); DVE derives the fp8 hi/lo
                        # pair. Act's double-gelu (~1.5us/tile with overheads)
                        # would exceed PE's 1.28us/tile and throttle mm1.
                        hbf = pfd.tile([128, CBLK], BF16, tag="hbf")
                        nc.scalar.activation(hbf[:], ps1[:], AF.Gelu, scale=INV_WSC)
                        nc.vector.tensor_copy(hT_h[:, ft, :], hbf[:])
                        nc.vector.tensor_tensor(
                            out=hT_l[:, ft, :], in0=hbf[:], in1=hT_h[:, ft, :],
                            op=OP.subtract,
                        )

                    if cb == 0:
                        # part zero-fill, entirely deferred into the FFN: a
                        # tiny release DMA (gated on cb0's dispatch, so it
                        # cannot start before the FFN does) opens a 1MB
                        # self-chained chunk chain over partL then partR.
                        # Hosted on SP, which is idle mid-FFN; the chain
                        # self-yields the FIFO, so dispatch gathers never wait
                        # more than one chunk. The eo scatters' WAW on the
                        # zero window orders them after their half's chunks.
                        ZA = T // 4   # partA chunk rows (1MB at DA cols)
                        ZB = T // 8   # partB chunk rows (1.5MB at DB cols)
                        zsrc_flat = zsrc_d.rearrange("a b -> (a b)")
                        nc.sync.dma_start(partA_d[0:1, 0:2], next_dispT[0:1, 0, 0:2])
                        nc.sync.dma_start(
                            partA_d[0:ZA, :],
                            zsrc_flat[0 : ZA * DA].rearrange("(r c) -> r c", c=DA),
                        )
                        for zc in range(1, 4):
                            nc.sync.dma_start(
                                partA_d[ZA * zc : ZA * (zc + 1), :],
                                partA_d[ZA * (zc - 1) : ZA * zc, :],
                            )
                        partA_flat = partA_d.rearrange("a b -> (a b)")
                        nc.sync.dma_start(
                            partB_d[0:ZB, :],
                            partA_flat[0 : ZB * DB].rearrange("(r c) -> r c", c=DB),
                        )
                        for zc in range(1, 8):
                            nc.sync.dma_start(
                                partB_d[ZB * zc : ZB * (zc + 1), :],
                                partB_d[ZB * (zc - 1) : ZB * zc, :],
                            )

                    # prefetch the next block's dispatch AFTER this block's mm1
                    # emission: Tile's in-order PE queue then runs those
                    # transposes only when their gathers are long done, instead
                    # of idling PE mid-mm1 waiting for them
                    if cb + 1 < NCB:
                        next_dispT = emit_dispatch(cb + 1)

                    # mm2: eo[c, d] = hT.T @ w_down (3 DR terms as in mm1).
                    # PSUM->SBUF copy applies the slot gate, which carries the
                    # 1/WSC factor from the payload pack. Each slot row is
                    # scattered as two contiguous spans — cols [0:DA) into
                    # partA, [DA:D) into partB (both at their tensor's column
                    # origin: the indirect scatter's static window AP must
                    # have offset 0) — with trash row T for empty slots.
                    # Actual rows come from the dynamic offsets; the cost
                    # model keys on the static 128-row window AP. The LAST
                    # block computes the DA columns of all its slots FIRST
                    # (256-wide tiles, column-outer) so the small partA
                    # ReduceScatter starts ~10us into the block while the big
                    # partB one is bounded by PE-end anyway.
                    eo_sb = pf.tile([128, CBLK // 128, D], BF16, tag="eo_sb")
                    last = cb == NCB - 1

                    def mm2_tile(ct, col0, width):
                        ps2 = psm.tile([128, 512], F32, space="PSUM", tag="ps2")
                        kk = 0
                        for hsb, vsb in ((hT_h, wdnh_sb), (hT_h, wdnl_sb), (hT_l, wdnh_sb)):
                            for ft2 in range(F // 256):
                                nc.tensor.matmul(
                                    ps2[:, 0:width],
                                    lhsT=hsb[:, 2 * ft2 : 2 * ft2 + 2, 128 * ct : 128 * ct + 128],
                                    rhs=vsb[:, 2 * ft2 : 2 * ft2 + 2, col0 : col0 + width],
                                    start=(kk == 0), stop=(kk == 3 * (F // 256) - 1),
                                    perf_mode=DR,
                                )
                                kk += 1
                        k = (CBLK // 128) * cb + ct
                        # eo drain on Act (idle during mm2): on DVE these
                        # queue behind the next block's dispatch copies and
                        # splits, delaying ps2 recycling and stalling PE
                        nc.scalar.activation(
                            eo_sb[:, ct, col0 : col0 + width], ps2[:, 0:width],
                            AF.Copy, scale=slotg[:, k : k + 1],
                        )

                    def scatter(pd, cols, ct, col0):
                        k = (CBLK // 128) * cb + ct
                        nc.gpsimd.indirect_dma_start(
                            out=pd[0:T, :].rearrange("(a b) d -> a (b d)", b=64)[:, 0:cols],
                            out_offset=bass.IndirectOffsetOnAxis(ap=tokc[:, k : k + 1], axis=0),
                            in_=eo_sb[:, ct, col0 : col0 + cols], in_offset=None,
                        )

                    if not last:
                        for ct in range(CBLK // 128):
                            for dc in range(D // 512):
                                mm2_tile(ct, 512 * dc, 512)
                            scatter(partA_d, DA, ct, 0)
                            scatter(partB_d, DB, ct, DA)
                    else:
                        # column-outer: DA-cols of every slot first, then the
                        # partA ReduceScatter, then the remaining columns
                        for ct in range(CBLK // 128):
                            mm2_tile(ct, 0, DA)
                            scatter(partA_d, DA, ct, 0)
                        nc.gpsimd.collective_compute(
                            "ReduceScatter", OP.add,
                            replica_groups=[list(range(NC))],
                            ins=[partA_d[0:T, :].opt()], outs=[rsA_out[:].opt()],
                        )
                        nc.sync.dma_start(y_d[:, 0:DA], rsA_out[:])
                        for ct in range(CBLK // 128):
                            mm2_tile(ct, DA, 512 - DA)
                            mm2_tile(ct, 512, 512)
                            scatter(partB_d, DB, ct, DA)

            # =============== COMBINE: big-half ReduceScatter ===============
            # (the small half was issued inside the last mm2 block; collectives
            # may not read or write IO tensors: internal in/out, then small
            # DMAs move the reduced shards to y — the partA bounce overlaps
            # this collective)
            nc.gpsimd.collective_compute(
                "ReduceScatter", OP.add,
                replica_groups=[list(range(NC))],
                ins=[partB_d[0:T, :].opt()], outs=[rsB_out[:].opt()],
            )
            nc.sync.dma_start(y_d[:, DA:D], rsB_out[:])

    nc.compile()
    return nc


_PROGRAM = None


def _get_program():
    global _PROGRAM
    if _PROGRAM is None:
        _PROGRAM = _build_program()
    return _PROGRAM


def host_constants():
    p = np.arange(128)
    return {
        "ident": np.eye(128, dtype=np.float32),
        "slmat": (np.arange(128)[None, :] > p[:, None]).astype(np.float32),
        "tidx": (64 * p[:, None] + np.arange(64)[None, :]).astype(np.float32),
        "eidx": np.tile(np.arange(E, dtype=np.float32), (128, 1)),
        "carrym": np.tile(np.where(np.arange(E * 64) % 64 == 0, 0.0, 1.0).astype(np.float32), (128, 1)),
    }


def _make_in_maps(x, wg, w_gate, w_down):
    x = np.asarray(x, np.float32)
    wg_np = np.asarray(wg, np.float32)
    w_gate_np = np.asarray(w_gate, np.float32)
    w_down_np = np.asarray(w_down, np.float32)

    tokens = x.reshape(T, D)
    xb = np.zeros((T + 1, D), ml_dtypes.bfloat16)
    xb[:T] = tokens.astype(ml_dtypes.bfloat16)

    # shard m holds tokens [SH*m, SH*(m+1)); its xT columns are permuted so
    # that matmul tile position j = 128*tt + p corresponds to local token
    # u = 512*(p//64) + 64*tt + (p%64), making both the payload write (two
    # affine half-DMAs) and the all-core reread (one affine DMA) contiguous.
    j = np.arange(SH)
    tt_, p_ = j // 128, j % 128
    perm = 512 * (p_ // 64) + 64 * tt_ + (p_ % 64)  # local token at column j
    consts = host_constants()
    p = np.arange(128)
    kk = np.arange(C // 128)

    F8NP = ml_dtypes.float8_e4m3

    def hilo(w):
        ws = (WSC * w).astype(np.float32)
        wh = ws.astype(F8NP)
        wl = (ws - wh.astype(np.float32)).astype(F8NP)
        return np.ascontiguousarray(wh), np.ascontiguousarray(wl)

    in_maps = []
    for m in range(NC):
        shard = tokens[SH * m : SH * (m + 1)]
        xT_sh = np.ascontiguousarray(shard[perm].T)
        wgth, wgtl = hilo(w_gate_np[m])
        wdnh, wdnl = hilo(w_down_np[m])
        in_maps.append({
            "xT_sh": xT_sh,
            "xb": xb,
            "wg": wg_np,
            "wgth": wgth,
            "wgtl": wgtl,
            "wdnh": wdnh,
            "wdnl": wdnl,
            "cid": np.full((128, 1), float(m), np.float32),
            "slotid": (m * C + 128 * kk[None, :] + p[:, None]).astype(np.float32),
            "zsrc": np.zeros((SH, D), ml_dtypes.bfloat16),
            **consts,
        })
    return in_maps


def kernel(x, wg, w_gate, w_down, _trace=False):
    global LAST_RESULT
    x = np.asarray(x, np.float32)
    in_maps = _make_in_maps(x, wg, w_gate, w_down)

    nc = _get_program()
    res = run_bass_kernel_spmd(nc, in_maps, core_ids=list(range(NC)), trace=_trace)
    LAST_RESULT = res
    out = np.concatenate([res.results[m]["y"] for m in range(NC)], axis=0)
    return out.reshape(B, S, D).astype(x.dtype)


def bench(x, wg, w_gate, w_down, iters=6):
    """Measure per-execution wall time with device-resident inputs.

    Returns (output, per_call_seconds_list) where each call gets freshly
    zeroed (donated) output buffers, matching run_bass_via_pjrt semantics.
    """
    import time
    import jax
    from jax.sharding import Mesh, PartitionSpec, NamedSharding
    from jax.experimental.shard_map import shard_map
    import concourse.mybir as _mybir
    from concourse.bass2jax import _bass_exec_p, install_neuronx_cc_hook, partition_id_tensor

    install_neuronx_cc_hook()
    nc = _get_program()

    x = np.asarray(x, np.float32)
    in_maps = _make_in_maps(x, wg, w_gate, w_down)

    in_names, out_names, out_avals, zero_outs = [], [], [], []
    for alloc in nc.m.functions[0].allocations:
        if not isinstance(alloc, _mybir.MemoryLocationSet):
            continue
        name = alloc.memorylocations[0].name
        if alloc.kind == "ExternalInput":
            if nc.partition_id_tensor is None or name != nc.partition_id_tensor.name:
                in_names.append(name)
        elif alloc.kind == "ExternalOutput":
            shape = tuple(alloc.tensor_shape)
            dtype = _mybir.dt.np(alloc.dtype)
            out_names.append(name)
            out_avals.append(jax.core.ShapedArray(shape, dtype))
            zero_outs.append(np.zeros(shape, dtype))
    n_params = len(in_names)
    all_in_names = in_names + out_names
    if nc.partition_id_tensor is not None:
        all_in_names = all_in_names + [nc.partition_id_tensor.name]

    def _body(*args):
        operands = list(args)
        if nc.partition_id_tensor is not None:
            operands.append(partition_id_tensor())
        outs = _bass_exec_p.bind(
            *operands,
            out_avals=tuple(out_avals),
            in_names=tuple(all_in_names),
            out_names=tuple(out_names),
            lowering_input_output_aliases=(),
            sim_require_finite=True,
            sim_require_nnan=True,
            nc=nc,
        )
        return tuple(outs)

    devices = jax.devices()[:NC]
    mesh = Mesh(np.asarray(devices), ("core",))
    nsh = NamedSharding(mesh, PartitionSpec("core"))
    n_outs = len(out_avals)
    donate = tuple(range(n_params, n_params + n_outs))
    sharded = jax.jit(
        shard_map(_body, mesh=mesh, in_specs=(PartitionSpec("core"),) * (n_params + n_outs),
                  out_specs=(PartitionSpec("core"),) * n_outs, check_rep=False),
        donate_argnums=donate, keep_unused=True,
    )

    concat_in = [
        jax.device_put(np.concatenate([np.asarray(in_maps[c][nm]) for c in range(NC)], axis=0), nsh)
        for nm in in_names
    ]
    zero_sets = [
        [jax.device_put(np.zeros((NC * z.shape[0], *z.shape[1:]), z.dtype), nsh) for z in zero_outs]
        for _ in range(iters + 1)
    ]

    out = sharded(*concat_in, *zero_sets[0])  # warmup + compile
    jax.block_until_ready(out)
    times = []
    for it in range(iters):
        t0 = time.perf_counter()
        out = sharded(*concat_in, *zero_sets[it + 1])
        jax.block_until_ready(out)
        times.append(time.perf_counter() - t0)

    outs = {
        nm: np.asarray(out[i]).reshape(NC, *out_avals[i].shape) for i, nm in enumerate(out_names)
    }
    y = np.concatenate([outs["y"][m] for m in range(NC)], axis=0).reshape(B, S, D).astype(x.dtype)
    return y, times

